# revision 1
# baseline (speedup 1.0000x reference)
"""Trainium2 Bass kernel for causal multi-head attention block (B=8, S=1024, D=1024, H=16).

Sharding: pure batch data-parallelism - one batch element per NeuronCore (B=8, 8 cores).
Each core runs the full transformer block on its [S, D] slice; no collectives.

Per-core algorithm (layouts chosen so no on-device transposes are needed):
  - Host passes x^T and all W^T pre-strided into the SBUF partition layout
    [p, db, cols] so every big DMA is 128 large contiguous descriptors.
  - QKV projections run in fp8e4 DoubleRow mode (two 128-deep k-tiles per pass,
    so K=256 per matmul at bf16-rate): host scales x by 8 and Wq/Wk/Wv by 256
    (keeps U(-1/32,1/32) weights out of fp8 denormals); the PSUM drain rescales
    by 1/2048 and adds the bias.
  - Q^T, K^T land as [o, s] bf16; per-head [dk, S] slices feed the scores matmul.
  - V lands natural [s, o] in fp8 per head with an appended ones column
    (V' = [V | padmask]) so the ctx matmul also yields the softmax denominator
    s0 in PSUM row 64.  Padded keys have V rows zeroed.
  - scoresT[k, q] = matmul(lhsT=K^T head slice, rhs=Q^T head slice) in bf16
    (contraction dk=64; even/odd heads at PE row groups 0/64 run concurrently).
    k-blocks processed in pairs sharing one 2-bank PSUM tile.
  - Softmax without max-subtraction: u = exp(0.125*scores) on ACT, written as
    fp8.  Causal masking multiplies u by 0/1 masks on DVE: a [128,128] triangle
    for the pair's lower block, and a [128,256] [zeros|triangle] extended mask
    for the upper block (which shares the lower block's column window, so its
    below-diagonal garbage must be zeroed for the DoubleRow ctx matmul).
  - ctx matmul in fp8 DoubleRow over k-block pairs (K=256 per pass):
    lhsT=V' pair, rhs=u pair.
  - Rows with a fully-masked causal window (s0 == 0) must match the reference's
    softmax(-1e9*ones) = uniform over ALL 1024 keys: ctx = (ctx_u + bad*sumV)/
    (s0 + 1024*bad), bad = (s0 <= 1e-30), sumV = column sums of V'.
  - The per-(head,q) normalizers are broadcast across partitions with a selector
    matmul (lhsT = 0/1 pair-selector, rhs = the 32-row table) into PSUM.
  - Out projection in bf16: out[s, o] = matmul(lhsT=ctx^T block, rhs=Wo^T);
    residual x+bo added, LayerNorm via bn_stats/bn_aggr.  gamma/beta applied on
    host (exact no-op for the reference's ones/zeros).
"""

import sys

import numpy as np

if "/opt/trn_rl_repo" not in sys.path:
    sys.path.insert(0, "/opt/trn_rl_repo")

S = 1024
D = 1024
H = 16
DK = 64
P = 128
DB = D // P  # 8 d-blocks
SB = S // P  # 8 s-blocks
NEG = -1.0e9
SCALE = 0.125  # 1/sqrt(64)
EPS = 1e-5
N_CORES = 8
XS = 8.0  # host scale on x for fp8
WS = 256.0  # host scale on Wq/Wk/Wv for fp8
INV_XW = 1.0 / (XS * WS)

_built = None


def _kbs(qc):
    """k-block pairs needed for q-chunk qc (q in [qc*512, qc*512+512))."""
    return [(0, 1), (2, 3)] if qc == 0 else [(0, 1), (2, 3), (4, 5), (6, 7)]


def _vs(kb, qc):
    """first causal q column within the 512-wide chunk for k-block kb."""
    return max(0, kb * P - qc * 512)


def _build():
    import concourse.mybir as mybir
    import concourse.tile as tile
    from concourse import bacc

    f32 = mybir.dt.float32
    bf16 = mybir.dt.bfloat16
    fp8 = mybir.dt.float8e4
    Alu = mybir.AluOpType
    Act = mybir.ActivationFunctionType
    DR = mybir.MatmulPerfMode.DoubleRow

    nc = bacc.Bacc()

    # ---- DRAM I/O (pre-strided [p, db, cols] contiguous layouts from host) ----
    xt_lo_d = nc.dram_tensor("xt_lo", [P, 4, S], fp8, kind="ExternalInput")
    xt_hi_d = nc.dram_tensor("xt_hi", [P, 4, S], fp8, kind="ExternalInput")
    xr_d = nc.dram_tensor("xr", [S, D], f32, kind="ExternalInput")  # x + bo
    w_d = {}
    for wname in ("wq", "wk", "wv", "wo"):
        for oc in range(2):
            n = f"{wname}{oc}"
            w_d[n] = nc.dram_tensor(n, [P, DB, 512], fp8, kind="ExternalInput")
    bqs_d = nc.dram_tensor("bqs", [P, DB], f32, kind="ExternalInput")
    bks_d = nc.dram_tensor("bks", [P, DB], f32, kind="ExternalInput")
    padm_d = nc.dram_tensor("padm", [P, SB], f32, kind="ExternalInput")  # 1 valid / 0 pad
    sumv_d = nc.dram_tensor("sumv", [P, DB], f32, kind="ExternalInput")
    pairsel_d = nc.dram_tensor("pairsel", [2 * H, SB * P], bf16, kind="ExternalInput")
    causal_d = nc.dram_tensor("causal", [P, P], fp8, kind="ExternalInput")  # 0/1
    causx_d = nc.dram_tensor("causx", [P, 2 * P], fp8, kind="ExternalInput")  # [0|tri]
    out_d = nc.dram_tensor("out", [S, D], f32, kind="ExternalOutput")

    with tile.TileContext(nc) as tc:
        with (
            tc.tile_pool(name="singles", bufs=1) as singles,
            tc.tile_pool(name="qt", bufs=1) as qt_pool,
            tc.tile_pool(name="kt", bufs=1) as kt_pool,
            tc.tile_pool(name="vp", bufs=1) as vp_pool,
            tc.tile_pool(name="xtp", bufs=2) as xtp,  # two halves of x^T (fp8)
            tc.tile_pool(name="wp", bufs=3) as wp,  # fp8 qkv weight chunks
            tc.tile_pool(name="up", bufs=6) as up,  # fp8 u chunks (2 live/iter)
            tc.tile_pool(name="wop", bufs=2) as wop,  # fp8 wo chunks
            tc.tile_pool(name="qt8", bufs=1) as qt8_pool,  # fp8 normalized ctx
            tc.tile_pool(name="xres", bufs=2) as xres_pool,
            tc.tile_pool(name="stg1", bufs=4) as stg1,
            tc.tile_pool(name="small", bufs=2) as small,
            tc.tile_pool(name="psmm", bufs=3, space="PSUM") as psmm,  # 2-bank tiles
            tc.tile_pool(name="psctx", bufs=2, space="PSUM") as psctx,
        ):
            # persistent big tensors; first weight chunk loads interleaved with x^T
            # so the first projection matmuls start as early as possible
            xT_lo = xtp.tile([P, 4, S], fp8, tag="xt", name="xT_lo")
            xT_hi = xtp.tile([P, 4, S], fp8, tag="xt", name="xT_hi")
            wch0 = wp.tile([P, DB, 512], fp8, tag="wp", name="wch0")
            nc.sync.dma_start(wch0[:, 0:4, :], w_d["wq0"][:, 0:4, :])
            for i in range(4):
                nc.sync.dma_start(xT_lo[:, i, :], xt_lo_d[:, i, :])
            nc.sync.dma_start(wch0[:, 4:8, :], w_d["wq0"][:, 4:8, :])
            for i in range(4):
                nc.sync.dma_start(xT_hi[:, i, :], xt_hi_d[:, i, :])

            def xT_pair(j, sl):
                """[P, 2, len] fp8 view of x^T k-tile pair j (db = 2j, 2j+1)."""
                t = xT_lo if j < 2 else xT_hi
                jj = j % 2
                return t[:, 2 * jj : 2 * jj + 2, sl]

            QT = qt_pool.tile([P, DB, S], bf16, tag="qt", name="QT")  # + ctx overlay
            KT = kt_pool.tile([P, DB, S], bf16, tag="kt", name="KT")
            QT8 = qt8_pool.tile([P, DB, S], fp8, tag="qt8", name="QT8")  # 8*ctx fp8
            VP_W = H * (DK + 1) + DK  # 64-col pad so head 15 has a 128-wide window
            Vp = vp_pool.tile([P, SB, VP_W], fp8, tag="vp", name="Vp")

            # ---- constants / singles ----
            bq_sb = singles.tile([P, DB], f32)
            nc.sync.dma_start(bq_sb[:], bqs_d[:, :])
            bk_sb = singles.tile([P, DB], f32)
            nc.sync.dma_start(bk_sb[:], bks_d[:, :])
            eps_sb = singles.tile([P, 1], f32)
            nc.vector.memset(eps_sb[:], EPS)
            ones_f32 = singles.tile([P, 1], f32)
            nc.vector.memset(ones_f32[:], 1.0)

            # epilogue table: cols 0:S s0 (later recip), S:2S bad (later bad*recip).
            # 32 partitions (rows 16-31 stay zero) so it is a clean K=32 rhs for the
            # selector broadcast matmuls.
            tab = singles.tile([2 * H, 2 * S], bf16)
            sumv_all = singles.tile([P, DB], f32)

            # pair-selector for PE-based partition-broadcast of tab rows:
            # pairsel[k, hb*128 + p] = 1 iff k == 2*hb + (p >= 64), host-provided
            pairsel = singles.tile([2 * H, SB * P], bf16)
            nc.sync.dma_start(pairsel[:], pairsel_d[:, :])
            # now zero the whole tab (rows 16-31 must stay zero; rows 0-15 get s0)
            nc.vector.tensor_scalar(
                tab[:, :],
                ones_f32[0 : 2 * H, 0:1].to_broadcast([2 * H, 2 * S]),
                0.0,
                None,
                op0=Alu.mult,
            )

            # ============ Phase 1: projections (fp8 DoubleRow, K=256/pass) ========
            # Q^T / K^T: psum[o_block 128, s 1024] = sum_j WT[pair j, ob].T @ xT[pair j]
            for wname, dst, bias_sb in (("wq", QT, bq_sb), ("wk", KT, bk_sb)):
                for oc in range(2):
                    if wname == "wq" and oc == 0:
                        wch = wch0
                    else:
                        wch = wp.tile([P, DB, 512], fp8, tag="wp", name="wch")
                        nc.sync.dma_start(wch[:, 0:4, :], w_d[f"{wname}{oc}"][:, 0:4, :])
                        nc.sync.dma_start(wch[:, 4:8, :], w_d[f"{wname}{oc}"][:, 4:8, :])
                    for obl in range(4):
                        ob = oc * 4 + obl
                        ps = psmm.tile([P, 2, 512], f32, tag="mm", name="ps_qk")
                        for sc in range(2):
                            for j in range(4):
                                nc.tensor.matmul(
                                    ps[:, sc, :],
                                    lhsT=wch[:, 2 * j : 2 * j + 2, obl * P : (obl + 1) * P],
                                    rhs=xT_pair(j, slice(sc * 512, (sc + 1) * 512)),
                                    start=(j == 0),
                                    stop=(j == 3),
                                    perf_mode=DR,
                                )
                        # rescale + per-partition bias (o on partitions), both chunks
                        nc.scalar.activation(
                            dst[:, ob, :],
                            ps[:].rearrange("p a b -> p (a b)"),
                            Act.Identity,
                            bias=bias_sb[:, ob : ob + 1],
                            scale=INV_XW,
                        )

            # late singles (not needed until V-proj / attention / epilogue)
            padm_sb = singles.tile([P, SB], f32)
            nc.sync.dma_start(padm_sb[:], padm_d[:, :])
            causal_sb = singles.tile([P, P], fp8)
            nc.sync.dma_start(causal_sb[:], causal_d[:, :])
            causx_sb = singles.tile([P, 2 * P], fp8)
            nc.sync.dma_start(causx_sb[:], causx_d[:, :])
            nc.sync.dma_start(sumv_all[:], sumv_d[:, :])
            # padm scaled by the fp8 descale factor, for the V drain
            padm_ds = singles.tile([P, SB], f32)
            nc.vector.tensor_scalar(padm_ds[:], padm_sb[:], INV_XW, None, op0=Alu.mult)
            # V natural: psum[s_block 128, o 512] = sum_j xT[pair j, sb].T @ WvT[pair j]
            for oc in range(2):
                wch = wp.tile([P, DB, 512], fp8, tag="wp", name="wchv")
                nc.sync.dma_start(wch[:, 0:4, :], w_d[f"wv{oc}"][:, 0:4, :])
                nc.sync.dma_start(wch[:, 4:8, :], w_d[f"wv{oc}"][:, 4:8, :])
                for sbi in range(0, SB, 2):
                    ps = psmm.tile([P, 2, 512], f32, tag="mm", name="ps_v")
                    for si in range(2):
                        sb = sbi + si
                        for j in range(4):
                            nc.tensor.matmul(
                                ps[:, si, :],
                                lhsT=xT_pair(j, slice(sb * P, (sb + 1) * P)),
                                rhs=wch[:, 2 * j : 2 * j + 2, :],
                                start=(j == 0),
                                stop=(j == 3),
                                perf_mode=DR,
                            )
                    for si in range(2):
                        sb = sbi + si
                        # scatter into per-head 65-wide slots; rescale + padmask
                        # (on ACT: out = in * scale(per-partition AP))
                        vview = Vp[:, sb, 0 : H * (DK + 1)].rearrange(
                            "p (h c) -> p h c", c=DK + 1
                        )
                        nc.scalar.activation(
                            vview[:, oc * 8 : (oc + 1) * 8, 0:DK],
                            ps[:, si, :].rearrange("p (h c) -> p h c", c=DK),
                            Act.Identity,
                            scale=padm_ds[:, sb : sb + 1],
                        )
            # "ones" columns of V' = padmask (zero for padded keys) + zeroed pad tail
            vv = Vp[:, :, 0 : H * (DK + 1)].rearrange("p sb (h c) -> p sb h c", c=DK + 1)
            nc.vector.tensor_copy(
                vv[:, :, :, DK : DK + 1],
                padm_sb.unsqueeze(2).unsqueeze(3).to_broadcast([P, SB, H, 1]),
            )
            nc.vector.tensor_scalar(
                Vp[:, :, H * (DK + 1) : VP_W],
                ones_f32.unsqueeze(1).to_broadcast([P, SB, DK]),
                0.0,
                None,
                op0=Alu.mult,
            )

            # ============ Phase 2: attention, qc-outer so the qc=0 epilogue +
            # first-half out-projection/LN overlap with qc=1 attention ========
            woch = []

            def emit_norm_out(qc):
                # ---- per-qc softmax normalization epilogue ----
                qch = slice(qc * 512, (qc + 1) * 512)
                T0 = tab[0:H, qc * 512 : (qc + 1) * 512]  # s0 -> denom -> recip
                T2 = tab[0:H, S + qc * 512 : S + (qc + 1) * 512]  # bad*1024
                nc.vector.tensor_scalar(
                    T2, T0, 1e-30, 1024.0, op0=Alu.is_le, op1=Alu.mult
                )
                nc.vector.tensor_tensor(T0, T0, T2, Alu.add)
                with nc.allow_low_precision(
                    reason="recip stored bf16; rounding far below output tolerance"
                ):
                    nc.vector.reciprocal(T0, T0)
                # fold the fp8 ctx scale (x8, dodges e4m3 denormals) into the recip
                nc.vector.tensor_scalar(T0, T0, 8.0, None, op0=Alu.mult)

                for hb in range(8):
                    bc2 = psmm.tile([P, 2, 512], f32, tag="mm", name="bc2")
                    nc.tensor.matmul(
                        bc2[:, 0, :],
                        lhsT=pairsel[:, hb * P : (hb + 1) * P],
                        rhs=tab[:, S + qc * 512 : S + (qc + 1) * 512],
                        start=True,
                        stop=True,
                    )
                    nc.tensor.matmul(
                        bc2[:, 1, :],
                        lhsT=pairsel[:, hb * P : (hb + 1) * P],
                        rhs=tab[:, qc * 512 : (qc + 1) * 512],
                        start=True,
                        stop=True,
                    )
                    bcp = bc2[:, 0, :]
                    rc = bc2[:, 1, :]
                    # ctx = (ctx_u + bad1024 * sumV/1024) * recip (whole pair)
                    nc.vector.scalar_tensor_tensor(
                        QT[:, hb, qch],
                        bcp,
                        sumv_all[:, hb : hb + 1],
                        QT[:, hb, qch],
                        op0=Alu.mult,
                        op1=Alu.add,
                    )
                    nc.vector.tensor_tensor(
                        QT8[:, hb, qch], QT[:, hb, qch], rc, Alu.mult
                    )

                # ---- out-projection + residual + LayerNorm for this q-half ----
                for sb in range(qc * 4, qc * 4 + 4):
                    xres = xres_pool.tile([P, D], f32, tag="xres", name="xres")
                    nc.sync.dma_start(xres[:], xr_d[sb * P : (sb + 1) * P, :])
                    res = xres  # residual-add and LN happen in place
                    ps = psmm.tile([P, 2, 512], f32, tag="mm", name="ps_o")
                    for oc in range(2):
                        for j in range(4):
                            nc.tensor.matmul(
                                ps[:, oc, :],
                                lhsT=QT8[:, 2 * j : 2 * j + 2, sb * P : (sb + 1) * P],
                                rhs=woch[oc][:, 2 * j : 2 * j + 2, :],
                                start=(j == 0),
                                stop=(j == 3),
                                perf_mode=DR,
                            )
                    # descale (ctx x8, Wo x256) + residual in one pass
                    nc.vector.scalar_tensor_tensor(
                        res[:, :],
                        ps[:].rearrange("p a b -> p (a b)"),
                        1.0 / (8.0 * WS),
                        xres[:, :],
                        op0=Alu.mult,
                        op1=Alu.add,
                    )
                    # LayerNorm over free dim (1024) via bn_stats (2 subgroups)
                    stats = small.tile([P, 2, 6], f32, tag="stats", name="stats")
                    nc.vector.bn_stats(stats[:, 0, :], res[:, 0:512])
                    nc.vector.bn_stats(stats[:, 1, :], res[:, 512:1024])
                    mv = small.tile([P, 2], f32, tag="mv", name="mv")
                    nc.vector.bn_aggr(mv[:], stats[:])
                    rstd = small.tile([P, 1], f32, tag="rstd", name="rstd")
                    nc.scalar.activation(
                        rstd[:], mv[:, 1:2], Act.Sqrt, bias=eps_sb[:], scale=1.0
                    )
                    nc.vector.reciprocal(rstd[:], rstd[:])
                    nc.vector.tensor_scalar(
                        res[:], res[:], mv[:, 0:1], rstd[:],
                        op0=Alu.subtract, op1=Alu.mult,
                    )
                    nc.sync.dma_start(out_d[sb * P : (sb + 1) * P, :], res[:])

            for qc in range(2):
                for hb in range(8):
                    uts = {}
                    for par in range(2):
                        ut = up.tile([P, DB, 512], fp8, tag="up", name=f"ut{par}")
                        uts[par] = ut
                    for kb0, kb1 in _kbs(qc):
                        vs = _vs(kb0, qc)  # pair shares the lower block's start col
                        for par in range(2):
                            hp = 64 * par
                            ps = psmm.tile([P, 2, 512], f32, tag="mm", name="ps_sc")
                            for i, kb in enumerate((kb0, kb1)):
                                nc.tensor.matmul(
                                    ps[:, i, vs:512],
                                    lhsT=KT[hp : hp + DK, hb, kb * P : (kb + 1) * P],
                                    rhs=QT[
                                        hp : hp + DK, hb, qc * 512 + vs : qc * 512 + 512
                                    ],
                                    start=True,
                                    stop=True,
                                )
                            # u = exp(0.125*scores); padding handled by zeroed V rows
                            upair = uts[par][:, kb0 : kb0 + 2, vs:512]
                            nc.scalar.activation(
                                upair, ps[:, :, vs:512], Act.Exp, scale=SCALE
                            )
                            if kb0 * P >= qc * 512:  # diagonal region: mask on u
                                # lower block: 0/1 triangle on its crossing square
                                nc.vector.tensor_mul(
                                    uts[par][:, kb0, vs : vs + P],
                                    uts[par][:, kb0, vs : vs + P],
                                    causal_sb[:],
                                )
                                # upper block: [zeros|triangle] over its garbage
                                # window + crossing square (it shares vs of kb0)
                                nc.vector.tensor_mul(
                                    uts[par][:, kb1, vs : vs + 2 * P],
                                    uts[par][:, kb1, vs : vs + 2 * P],
                                    causx_sb[:],
                                )
                    for par in range(2):
                        h = 2 * hb + par
                        cps = psctx.tile([P, 512], f32, tag="ctx", name="cps")
                        pairs = _kbs(qc)
                        for i, (kb0, kb1) in enumerate(pairs):
                            vs = _vs(kb0, qc)
                            nc.tensor.matmul(
                                cps[:, vs:512],
                                lhsT=Vp[:, kb0 : kb0 + 2, h * (DK + 1) : h * (DK + 1) + P],
                                rhs=uts[par][:, kb0 : kb0 + 2, vs:512],
                                start=(i == 0),
                                stop=(i == len(pairs) - 1),
                                perf_mode=DR,
                            )
                        # drain ctx + s0; s0 always staged via stg (a QT row 64
                        # staging spot would race the odd head's gpsimd write)
                        if par == 0:
                            nc.scalar.activation(
                                QT[0:DK, hb, qc * 512 : (qc + 1) * 512],
                                cps[0:DK, 0:512],
                                Act.Identity,
                            )
                            stge = stg1.tile([P, 512], bf16, tag="stg", name="stge")
                            nc.vector.tensor_copy(
                                stge[DK : DK + 1, 0:512], cps[DK : DK + 1, 0:512]
                            )
                            nc.sync.dma_start(
                                tab[h : h + 1, qc * 512 : (qc + 1) * 512],
                                stge[DK : DK + 1, 0:512],
                            )
                        else:
                            stg = stg1.tile([P, 512], bf16, tag="stg", name="stg")
                            nc.vector.tensor_copy(
                                stg[0 : DK + 1, 0:512], cps[0 : DK + 1, 0:512]
                            )
                            nc.sync.dma_start(
                                tab[h : h + 1, qc * 512 : (qc + 1) * 512],
                                stg[DK : DK + 1, 0:512],
                            )
                            nc.gpsimd.tensor_copy(
                                QT[DK:P, hb, qc * 512 : (qc + 1) * 512],
                                stg[0:DK, 0:512],
                            )

                if qc == 0:
                    # prefetch Wo chunks (used by the out-proj)
                    for oc in range(2):
                        wch = wop.tile([P, DB, 512], fp8, tag="wop", name="woch")
                        nc.sync.dma_start(wch[:, 0:4, :], w_d[f"wo{oc}"][:, 0:4, :])
                        nc.sync.dma_start(wch[:, 4:8, :], w_d[f"wo{oc}"][:, 4:8, :])
                        woch.append(wch)
                emit_norm_out(qc)

    nc.compile()
    return nc


def _stripe_w(WT):
    """[D, D] (d_in, d_out) -> two contiguous [P, DB, 512] o-half chunks."""
    a = np.ascontiguousarray(WT.reshape(DB, P, D).transpose(1, 0, 2))  # [p, db, o]
    return (
        np.ascontiguousarray(a[:, :, 0:512]),
        np.ascontiguousarray(a[:, :, 512:1024]),
    )


def kernel(
    history_items,
    sequence_mask,
    Wq,
    bq,
    Wk,
    bk,
    Wv,
    bv,
    Wo,
    bo,
    ln_gamma,
    ln_beta,
):
    from concourse.bass_utils import run_bass_kernel_spmd

    global _built
    if _built is None:
        _built = _build()
    nc = _built

    import ml_dtypes

    bf16 = ml_dtypes.bfloat16
    fp8 = ml_dtypes.float8_e4m3
    x = np.asarray(history_items, dtype=np.float32)
    mask = np.asarray(sequence_mask)
    f = lambda a: np.ascontiguousarray(np.asarray(a, dtype=np.float32))
    fb = lambda a: np.ascontiguousarray(np.asarray(a, dtype=np.float32).astype(bf16))
    f8 = lambda a: np.ascontiguousarray(np.asarray(a, dtype=np.float32).astype(fp8))

    common = {}
    for wname, W in (("wq", Wq), ("wk", Wk), ("wv", Wv), ("wo", Wo)):
        c0, c1 = _stripe_w(f(np.asarray(W).T * WS))
        common[f"{wname}0"] = f8(c0)
        common[f"{wname}1"] = f8(c1)
    common["bqs"] = f(np.asarray(bq).reshape(DB, P).T)
    common["bks"] = f(np.asarray(bk).reshape(DB, P).T)
    k_idx = np.arange(2 * H)[:, None]
    hb_idx = np.repeat(np.arange(SB), P)[None, :]
    c1_idx = np.tile((np.arange(P) >= 64).astype(np.int64), SB)[None, :]
    common["pairsel"] = fb((k_idx == 2 * hb_idx + c1_idx).astype(np.float32))
    tri = np.where(
        np.arange(P)[None, :] >= np.arange(P)[:, None], 1.0, 0.0
    ).astype(np.float32)
    common["causal"] = f8(tri)
    common["causx"] = f8(np.concatenate([np.zeros((P, P), np.float32), tri], axis=1))
    # attn-output bias bv contributes bv @ Wo.T (constant over s) -> fold into residual
    bo_row = (
        np.asarray(bo, dtype=np.float64)
        + np.asarray(bv, dtype=np.float64) @ np.asarray(Wo, dtype=np.float64).T
    ).astype(np.float32)

    in_maps = []
    for b in range(N_CORES):
        xT = f(x[b].T * XS).astype(fp8).reshape(DB, P, S).transpose(1, 0, 2)
        pm = (mask[b] != 0).astype(np.float32)
        sx = x[b].astype(np.float64).sum(axis=0)
        sumv = ((sx @ np.asarray(Wv, dtype=np.float64).T) / 1024.0).astype(np.float32)
        in_maps.append(
            {
                **common,
                "xt_lo": np.ascontiguousarray(xT[:, 0:4, :]),
                "xt_hi": np.ascontiguousarray(xT[:, 4:8, :]),
                "xr": f(x[b] + bo_row[None, :]),
                "padm": f(pm.reshape(SB, P).T),
                "sumv": f(sumv.reshape(DB, P).T),
            }
        )

    r = run_bass_kernel_spmd(nc, in_maps, core_ids=list(range(N_CORES)))
    out = np.stack([res["out"] for res in r.results]).astype(np.float32)

    g = np.asarray(ln_gamma, dtype=np.float32)
    be = np.asarray(ln_beta, dtype=np.float32)
    out = out * g[None, None, :] + be[None, None, :]
    return out.astype(np.float32)



# revision 6
# speedup vs baseline: 1.1098x; 1.1098x over previous
"""Trainium2 Bass kernel for causal multi-head attention block (B=8, S=1024, D=1024, H=16).

Sharding: pure batch data-parallelism - one batch element per NeuronCore (B=8, 8 cores).
Each core runs the full transformer block on its [S, D] slice; no collectives.

v2: restructured for ACT(exp)-bound pipelining.
  - Phase A interleaves per-head-block QK projections with qc0 attention and the
    below-diagonal half of qc1's scores+exp, so the Scalar engine's exp stream
    (the bottleneck) starts ~5us in and never starves, while the PE stream stays
    dense (keeps the HAM clock-gate at K=8/8).
  - QKV layouts / fp8 DoubleRow matmuls / softmax-without-max as in v1:
    host passes x^T and W^T pre-strided; QKV in fp8 DR (K=256/pass, host scales
    x by 8, W by 256); u = exp(0.125*scores) stored fp8; V' = [V | padmask] per
    head so the ctx matmul also yields the softmax denominator s0.
  - Bad-row (fully-masked causal window) handling is host-driven: badrow[q] =
    1024*(no valid key <= q) is an input, so the numerator correction is one
    K=1 matmul accumulated into the ctx chain (qc0 only; host asserts bad rows
    only occur for q<64) and the denominator fix is one [16,64] add on the s0
    table. This kills the is_le/scalar_tensor_tensor epilogue passes.
  - Normalization: s0 rows -> tab (bf16) -> reciprocal -> ONE selector-broadcast
    matmul per head-pair (8.0 fp8-descale folded into the selector values) ->
    one fused DVE multiply QT8 = QT * rc.
  - All PSUM drains on DVE (ACT does exp only); par1 ctx partition-shift via
    SBUF->SBUF DMA instead of gpsimd.
  - LayerNorm rstd: one batched Act.Rsqrt at the kernel tail (exactly one ACT
    table switch for the whole kernel).
"""

import sys

import numpy as np

if "/opt/trn_rl_repo" not in sys.path:
    sys.path.insert(0, "/opt/trn_rl_repo")

S = 1024
D = 1024
H = 16
DK = 64
P = 128
DB = D // P  # 8 d-blocks
SB = S // P  # 8 s-blocks
SCALE = 0.125  # 1/sqrt(64)
EPS = 1e-5
N_CORES = 8
XS = 8.0  # host scale on x for fp8
WS = 256.0  # host scale on Wq/Wk/Wv/Wo for fp8
INV_XW = 1.0 / (XS * WS)
CTX8 = 8.0  # fp8 scale on normalized ctx (folded into pairsel)

_built = None


def _build():
    import concourse.mybir as mybir
    import concourse.tile as tile
    from concourse import bacc

    f32 = mybir.dt.float32
    bf16 = mybir.dt.bfloat16
    fp8 = mybir.dt.float8e4
    Alu = mybir.AluOpType
    Act = mybir.ActivationFunctionType
    DR = mybir.MatmulPerfMode.DoubleRow

    nc = bacc.Bacc()

    # ---- DRAM I/O (pre-strided [p, db, cols] contiguous layouts from host) ----
    xt_lo_d = nc.dram_tensor("xt_lo", [P, 4, S], fp8, kind="ExternalInput")
    xt_hi_d = nc.dram_tensor("xt_hi", [P, 4, S], fp8, kind="ExternalInput")
    xr_d = nc.dram_tensor("xr", [S, D], f32, kind="ExternalInput")  # x + bo
    w_d = {}
    for wname in ("wq", "wk", "wv", "wo"):
        for oc in range(2):
            n = f"{wname}{oc}"
            w_d[n] = nc.dram_tensor(n, [P, DB, 512], fp8, kind="ExternalInput")
    bqs_d = nc.dram_tensor("bqs", [P, DB], f32, kind="ExternalInput")
    bks_d = nc.dram_tensor("bks", [P, DB], f32, kind="ExternalInput")
    padm_d = nc.dram_tensor("padm", [P, SB], f32, kind="ExternalInput")  # 1 valid / 0 pad
    pairsel_d = nc.dram_tensor("pairsel", [2 * H, SB * P], bf16, kind="ExternalInput")
    causal_d = nc.dram_tensor("causal", [P, P], fp8, kind="ExternalInput")  # 0/1
    causx_d = nc.dram_tensor("causx", [P, 2 * P], fp8, kind="ExternalInput")  # [0|tri]
    badrow_d = nc.dram_tensor("badrow", [H, 64], bf16, kind="ExternalInput")
    sumvr_d = nc.dram_tensor("sumvr", [1, H * (DK + 1)], bf16, kind="ExternalInput")
    out_d = nc.dram_tensor("out", [S, D], f32, kind="ExternalOutput")

    with tile.TileContext(nc) as tc:
        with (
            tc.tile_pool(name="singles", bufs=1) as singles,
            tc.tile_pool(name="qt", bufs=1) as qt_pool,
            tc.tile_pool(name="kt", bufs=1) as kt_pool,
            tc.tile_pool(name="vp", bufs=1) as vp_pool,
            tc.tile_pool(name="xtp", bufs=2) as xtp,
            tc.tile_pool(name="wp", bufs=4) as wp,  # fp8 qkv weight chunks
            tc.tile_pool(name="u0", bufs=4) as u0_pool,  # qc0 u tiles (rotating)
            tc.tile_pool(name="u1lo", bufs=16) as u1lo_pool,  # qc1 below-diag (held)
            tc.tile_pool(name="u1hi", bufs=4) as u1hi_pool,  # qc1 diag (rotating)
            tc.tile_pool(name="wop", bufs=2) as wop,
            tc.tile_pool(name="qt8", bufs=1) as qt8_pool,
            tc.tile_pool(name="xres", bufs=8) as xres_pool,  # all 8 live to tail
            tc.tile_pool(name="stg1", bufs=4) as stg1,
            tc.tile_pool(name="small", bufs=2) as small,
            tc.tile_pool(name="psq", bufs=2, space="PSUM") as psq,  # 1-bank tiles
            tc.tile_pool(name="psmm", bufs=2, space="PSUM") as psmm,  # 2-bank tiles
            tc.tile_pool(name="psctx", bufs=2, space="PSUM") as psctx,  # 1-bank
        ):
            # persistent big tensors; first weight chunk loads interleaved with x^T
            xT_lo = xtp.tile([P, 4, S], fp8, tag="xt", name="xT_lo")
            xT_hi = xtp.tile([P, 4, S], fp8, tag="xt", name="xT_hi")
            wq0 = wp.tile([P, DB, 512], fp8, tag="wp", name="wq0")
            nc.sync.dma_start(wq0[:, 0:4, :], w_d["wq0"][:, 0:4, :])
            for i in range(4):
                nc.sync.dma_start(xT_lo[:, i, :], xt_lo_d[:, i, :])
            nc.sync.dma_start(wq0[:, 4:8, :], w_d["wq0"][:, 4:8, :])
            for i in range(4):
                nc.sync.dma_start(xT_hi[:, i, :], xt_hi_d[:, i, :])
            wk0 = wp.tile([P, DB, 512], fp8, tag="wp", name="wk0")
            nc.sync.dma_start(wk0[:, 0:4, :], w_d["wk0"][:, 0:4, :])
            nc.sync.dma_start(wk0[:, 4:8, :], w_d["wk0"][:, 4:8, :])

            def xT_pair(j, sl):
                t = xT_lo if j < 2 else xT_hi
                jj = j % 2
                return t[:, 2 * jj : 2 * jj + 2, sl]

            QT = qt_pool.tile([P, DB, S], bf16, tag="qt", name="QT")  # + ctx overlay
            KT = kt_pool.tile([P, DB, S], bf16, tag="kt", name="KT")
            QT8 = qt8_pool.tile([P, DB, S], fp8, tag="qt8", name="QT8")  # 8*ctx fp8
            VP_W = H * (DK + 1) + DK  # 64-col pad so head 15 has a 128-wide window
            Vp = vp_pool.tile([P, SB, VP_W], fp8, tag="vp", name="Vp")

            # ---- constants / singles ----
            bq_sb = singles.tile([P, DB], f32)
            nc.sync.dma_start(bq_sb[:], bqs_d[:, :])
            bk_sb = singles.tile([P, DB], f32)
            nc.sync.dma_start(bk_sb[:], bks_d[:, :])
            eps_sb = singles.tile([P, 1], f32)
            nc.vector.memset(eps_sb[:], EPS)
            ones_f32 = singles.tile([P, 1], f32)
            nc.vector.memset(ones_f32[:], 1.0)
            padm_sb = singles.tile([P, SB], f32)
            nc.sync.dma_start(padm_sb[:], padm_d[:, :])
            causal_sb = singles.tile([P, P], fp8)
            nc.sync.dma_start(causal_sb[:], causal_d[:, :])
            causx_sb = singles.tile([P, 2 * P], fp8)
            nc.sync.dma_start(causx_sb[:], causx_d[:, :])
            badrow_sb = singles.tile([H, 64], bf16)
            nc.sync.dma_start(badrow_sb[:], badrow_d[:, :])
            sumvr_sb = singles.tile([1, H * (DK + 1)], bf16)
            nc.sync.dma_start(sumvr_sb[:], sumvr_d[:, :])
            pairsel = singles.tile([2 * H, SB * P], bf16)
            nc.sync.dma_start(pairsel[:], pairsel_d[:, :])
            # padm scaled by the fp8 descale factor, for the V drain
            padm_ds = singles.tile([P, SB], f32)
            nc.vector.tensor_scalar(padm_ds[:], padm_sb[:], INV_XW, None, op0=Alu.mult)

            # s0 table: row h, cols qc*512.. = softmax denominators (-> recip).
            # 32 partitions (rows 16-31 stay zero: clean K=32 rhs for bcast MMs).
            tab = singles.tile([2 * H, S], bf16)
            nc.vector.tensor_scalar(
                tab[:, :],
                ones_f32[0 : 2 * H, 0:1].to_broadcast([2 * H, S]),
                0.0,
                None,
                op0=Alu.mult,
            )
            # LN stats collected per s-block; batched Rsqrt at the tail
            mv_all = singles.tile([P, SB, 2], f32)
            rstd8 = singles.tile([P, SB], f32)

            # ================= emission helpers =================
            def proj_qk(hb):
                """Q^T,K^T projection for o-block hb (fp8 DR, drains on DVE)."""
                oc, obl = hb // 4, hb % 4
                wq = wqs[oc]
                wk = wks[oc]
                for wch, dst, bias_sb in ((wq, QT, bq_sb), (wk, KT, bk_sb)):
                    for sc in range(2):
                        ps = psq.tile([P, 512], f32, tag="q", name="ps_qk")
                        for j in range(4):
                            nc.tensor.matmul(
                                ps[:, :],
                                lhsT=wch[:, 2 * j : 2 * j + 2, obl * P : (obl + 1) * P],
                                rhs=xT_pair(j, slice(sc * 512, (sc + 1) * 512)),
                                start=(j == 0),
                                stop=(j == 3),
                                perf_mode=DR,
                            )
                        nc.vector.tensor_scalar(
                            dst[:, hb, sc * 512 : (sc + 1) * 512],
                            ps[:, :],
                            INV_XW,
                            bias_sb[:, hb : hb + 1],
                            op0=Alu.mult,
                            op1=Alu.add,
                        )

            def proj_v(oc):
                """V natural [s, o] for o-half oc into per-head 65-wide slots."""
                wch = wvs[oc]
                for sbi in range(0, SB, 2):
                    ps = psmm.tile([P, 2, 512], f32, tag="mm", name="ps_v")
                    for si in range(2):
                        sb = sbi + si
                        for j in range(4):
                            nc.tensor.matmul(
                                ps[:, si, :],
                                lhsT=xT_pair(j, slice(sb * P, (sb + 1) * P)),
                                rhs=wch[:, 2 * j : 2 * j + 2, :],
                                start=(j == 0),
                                stop=(j == 3),
                                perf_mode=DR,
                            )
                    for si in range(2):
                        sb = sbi + si
                        vview = Vp[:, sb, 0 : H * (DK + 1)].rearrange(
                            "p (h c) -> p h c", c=DK + 1
                        )
                        nc.scalar.activation(
                            vview[:, oc * 8 : (oc + 1) * 8, 0:DK],
                            ps[:, si, :].rearrange("p (h c) -> p h c", c=DK),
                            Act.Identity,
                            scale=padm_ds[:, sb : sb + 1],
                        )
                # ones columns of V' = padmask for this o-half's heads
                vv = Vp[:, :, 0 : H * (DK + 1)].rearrange(
                    "p sb (h c) -> p sb h c", c=DK + 1
                )
                nc.vector.tensor_copy(
                    vv[:, :, oc * 8 : (oc + 1) * 8, DK : DK + 1],
                    padm_sb.unsqueeze(2).unsqueeze(3).to_broadcast([P, SB, 8, 1]),
                )
                if oc == 1:  # zero the pad tail once
                    nc.vector.tensor_scalar(
                        Vp[:, :, H * (DK + 1) : VP_W],
                        ones_f32.unsqueeze(1).to_broadcast([P, SB, DK]),
                        0.0,
                        None,
                        op0=Alu.mult,
                    )

            def scores_exp(hb, par, kb0, kb1, vs, qc, ut, ut_kb_base, masks):
                """Scores matmul pair + exp (+ causal masks if masks)."""
                hp = 64 * par
                q0 = qc * 512
                ps = psmm.tile([P, 2, 512], f32, tag="mm", name="ps_sc")
                for i, kb in enumerate((kb0, kb1)):
                    nc.tensor.matmul(
                        ps[:, i, vs:512],
                        lhsT=KT[hp : hp + DK, hb, kb * P : (kb + 1) * P],
                        rhs=QT[hp : hp + DK, hb, q0 + vs : q0 + 512],
                        start=True,
                        stop=True,
                    )
                k0 = kb0 - ut_kb_base
                upair = ut[:, k0 : k0 + 2, vs:512]
                nc.scalar.activation(upair, ps[:, :, vs:512], Act.Exp, scale=SCALE)
                if masks:
                    nc.vector.tensor_mul(
                        ut[:, k0, vs : vs + P],
                        ut[:, k0, vs : vs + P],
                        causal_sb[:],
                    )
                    nc.vector.tensor_mul(
                        ut[:, k0 + 1, vs : vs + 2 * P],
                        ut[:, k0 + 1, vs : vs + 2 * P],
                        causx_sb[:],
                    )

            def ctx_and_drain(hb, qc, rhs_of_pair):
                """ctx DR chains for both heads of hb + s0/ctx drains.

                rhs_of_pair(par, kb0) -> (u_ap) for that pair; pairs/vs per qc.
                qc0 additionally accumulates the host-driven bad-row correction.
                """
                qch = slice(qc * 512, (qc + 1) * 512)
                pairs = [(0, 1), (2, 3)] if qc == 0 else [(0, 1), (2, 3), (4, 5), (6, 7)]
                for par in range(2):
                    h = 2 * hb + par
                    cps = psctx.tile([P, 512], f32, tag="ctx", name="cps")
                    for i, (kb0, kb1) in enumerate(pairs):
                        vs = max(0, kb0 * P - qc * 512)
                        nc.tensor.matmul(
                            cps[:, vs:512],
                            lhsT=Vp[:, kb0 : kb0 + 2, h * (DK + 1) : h * (DK + 1) + P],
                            rhs=rhs_of_pair(par, kb0)[:, :, vs:512],
                            start=(i == 0),
                            stop=(qc == 1 and i == len(pairs) - 1),
                            perf_mode=DR,
                        )
                    if qc == 0:
                        # numerator bad-row fix: ctx += sumv[d] * badrow1024[q]
                        nc.tensor.matmul(
                            cps[0 : DK + 1, 0:64],
                            lhsT=sumvr_sb[0:1, h * (DK + 1) : (h + 1) * (DK + 1)],
                            rhs=badrow_sb[0:1, :],
                            start=False,
                            stop=True,
                        )
                    if par == 0:
                        # ctx rows already partition-aligned: direct DVE drain
                        nc.vector.tensor_copy(QT[0:DK, hb, qch], cps[0:DK, 0:512])
                        stge = stg1.tile([P, 512], bf16, tag="stg", name="stge")
                        nc.vector.tensor_copy(
                            stge[DK : DK + 1, 0:512], cps[DK : DK + 1, 0:512]
                        )
                        nc.sync.dma_start(tab[h : h + 1, qch], stge[DK : DK + 1, 0:512])
                    else:
                        stg = stg1.tile([P, 512], bf16, tag="stg", name="stg")
                        nc.vector.tensor_copy(
                            stg[0 : DK + 1, 0:512], cps[0 : DK + 1, 0:512]
                        )
                        nc.sync.dma_start(tab[h : h + 1, qch], stg[DK : DK + 1, 0:512])
                        # partition shift 0:64 -> 64:128 via SBUF->SBUF DMA
                        nc.sync.dma_start(QT[DK:P, hb, qch], stg[0:DK, 0:512])

            def attn_qc0(hb):
                uts = {
                    par: u0_pool.tile([P, 4, 512], fp8, tag="u0", name=f"u0_{par}")
                    for par in range(2)
                }
                for kb0, kb1 in ((0, 1), (2, 3)):
                    vs = kb0 * P
                    for par in range(2):
                        scores_exp(hb, par, kb0, kb1, vs, 0, uts[par], 0, True)
                ctx_and_drain(hb, 0, lambda par, kb0: uts[par][:, kb0 : kb0 + 2, :])

            def qc1_lower(hb):
                for par in range(2):
                    ut = u1lo_pool.tile(
                        [P, 4, 512], fp8, tag="u1lo", name=f"u1lo_{hb}_{par}"
                    )
                    u1lo[(hb, par)] = ut
                    for kb0, kb1 in ((0, 1), (2, 3)):
                        scores_exp(hb, par, kb0, kb1, 0, 1, ut, 0, False)

            def qc1_upper(hb):
                uts = {
                    par: u1hi_pool.tile([P, 4, 512], fp8, tag="u1hi", name=f"u1hi_{par}")
                    for par in range(2)
                }
                for kb0, kb1 in ((4, 5), (6, 7)):
                    vs = kb0 * P - 512
                    for par in range(2):
                        scores_exp(hb, par, kb0, kb1, vs, 1, uts[par], 4, True)

                def rhs(par, kb0):
                    if kb0 < 4:
                        return u1lo[(hb, par)][:, kb0 : kb0 + 2, :]
                    return uts[par][:, kb0 - 4 : kb0 - 4 + 2, :]

                ctx_and_drain(hb, 1, rhs)

            def epilogue(qc):
                qch = slice(qc * 512, (qc + 1) * 512)
                T0 = tab[0:H, qch]
                if qc == 0:
                    # denominator bad-row fix: s0 += 1024*bad  (bad only in q<64)
                    nc.vector.tensor_tensor(
                        tab[0:H, 0:64], tab[0:H, 0:64], badrow_sb[:, :], Alu.add
                    )
                with nc.allow_low_precision(
                    reason="recip stored bf16; rounding far below output tolerance"
                ):
                    nc.vector.reciprocal(T0, T0)
                for hb in range(8):
                    rc = psq.tile([P, 512], f32, tag="q", name="rc")
                    nc.tensor.matmul(
                        rc[:, :],
                        lhsT=pairsel[:, hb * P : (hb + 1) * P],
                        rhs=tab[:, qch],
                        start=True,
                        stop=True,
                    )
                    nc.vector.tensor_tensor(
                        QT8[:, hb, qch], QT[:, hb, qch], rc[:, :], Alu.mult
                    )

            def outproj(qc):
                for sb in range(qc * 4, qc * 4 + 4):
                    xres = xres_list[sb]
                    ps = psmm.tile([P, 2, 512], f32, tag="mm", name="ps_o")
                    for oc in range(2):
                        for j in range(4):
                            nc.tensor.matmul(
                                ps[:, oc, :],
                                lhsT=QT8[:, 2 * j : 2 * j + 2, sb * P : (sb + 1) * P],
                                rhs=wos[oc][:, 2 * j : 2 * j + 2, :],
                                start=(j == 0),
                                stop=(j == 3),
                                perf_mode=DR,
                            )
                    # descale (ctx x8, Wo x256) + residual in one pass
                    nc.vector.scalar_tensor_tensor(
                        xres[:, :],
                        ps[:].rearrange("p a b -> p (a b)"),
                        1.0 / (CTX8 * WS),
                        xres[:, :],
                        op0=Alu.mult,
                        op1=Alu.add,
                    )
                    stats = small.tile([P, 2, 6], f32, tag="stats", name="stats")
                    nc.vector.bn_stats(stats[:, 0, :], xres[:, 0:512])
                    nc.vector.bn_stats(stats[:, 1, :], xres[:, 512:1024])
                    nc.vector.bn_aggr(mv_all[:, sb, :], stats[:])

            # ================= emission =================
            u1lo = {}
            wqs, wks, wvs, wos = {0: wq0}, {0: wk0}, {}, []

            # Phase A: projections interleaved with qc0 attention + qc1-lower
            for hb in range(8):
                if hb == 0:
                    wvs[0] = wp.tile([P, DB, 512], fp8, tag="wp", name="wv0")
                    nc.sync.dma_start(wvs[0][:, 0:4, :], w_d["wv0"][:, 0:4, :])
                    nc.sync.dma_start(wvs[0][:, 4:8, :], w_d["wv0"][:, 4:8, :])
                if hb == 1:
                    wvs[1] = wp.tile([P, DB, 512], fp8, tag="wp", name="wv1")
                    nc.sync.dma_start(wvs[1][:, 0:4, :], w_d["wv1"][:, 0:4, :])
                    nc.sync.dma_start(wvs[1][:, 4:8, :], w_d["wv1"][:, 4:8, :])
                if hb == 2:
                    wqs[1] = wp.tile([P, DB, 512], fp8, tag="wp", name="wq1")
                    nc.sync.dma_start(wqs[1][:, 0:4, :], w_d["wq1"][:, 0:4, :])
                    nc.sync.dma_start(wqs[1][:, 4:8, :], w_d["wq1"][:, 4:8, :])
                    wks[1] = wp.tile([P, DB, 512], fp8, tag="wp", name="wk1")
                    nc.sync.dma_start(wks[1][:, 0:4, :], w_d["wk1"][:, 0:4, :])
                    nc.sync.dma_start(wks[1][:, 4:8, :], w_d["wk1"][:, 4:8, :])
                proj_qk(hb)
                if hb == 0:
                    proj_v(0)
                if hb == 1:
                    proj_v(1)
                attn_qc0(hb)
                qc1_lower(hb)
                if hb == 6:
                    for oc in range(2):
                        wch = wop.tile([P, DB, 512], fp8, tag="wop", name="woch")
                        nc.sync.dma_start(wch[:, 0:4, :], w_d[f"wo{oc}"][:, 0:4, :])
                        nc.sync.dma_start(wch[:, 4:8, :], w_d[f"wo{oc}"][:, 4:8, :])
                        wos.append(wch)

            # residual inputs for all 8 s-blocks (also the LN/output buffers)
            xres_list = []
            for sb in range(SB):
                xres = xres_pool.tile([P, D], f32, tag="xres", name=f"xres{sb}")
                nc.sync.dma_start(xres[:], xr_d[sb * P : (sb + 1) * P, :])
                xres_list.append(xres)

            # Phase B: qc1 diagonal attention, epilogues, out-projections
            qc1_upper(0)
            qc1_upper(1)
            epilogue(0)
            qc1_upper(2)
            qc1_upper(3)
            outproj(0)
            qc1_upper(4)
            qc1_upper(5)
            qc1_upper(6)
            qc1_upper(7)
            epilogue(1)
            outproj(1)

            # Tail: batched LayerNorm finish (one ACT table switch total)
            nc.scalar.activation(
                rstd8[:, :], mv_all[:, :, 1:2], Act.Sqrt, bias=eps_sb[:], scale=1.0
            )
            nc.vector.reciprocal(rstd8[:, :], rstd8[:, :])
            for sb in range(SB):
                xres = xres_list[sb]
                nc.vector.tensor_scalar(
                    xres[:],
                    xres[:],
                    mv_all[:, sb, 0:1],
                    rstd8[:, sb : sb + 1],
                    op0=Alu.subtract,
                    op1=Alu.mult,
                )
                nc.sync.dma_start(out_d[sb * P : (sb + 1) * P, :], xres[:])

    nc.compile()
    return nc


def _stripe_w(WT):
    """[D, D] (d_in, d_out) -> two contiguous [P, DB, 512] o-half chunks."""
    a = np.ascontiguousarray(WT.reshape(DB, P, D).transpose(1, 0, 2))  # [p, db, o]
    return (
        np.ascontiguousarray(a[:, :, 0:512]),
        np.ascontiguousarray(a[:, :, 512:1024]),
    )


def kernel(
    history_items,
    sequence_mask,
    Wq,
    bq,
    Wk,
    bk,
    Wv,
    bv,
    Wo,
    bo,
    ln_gamma,
    ln_beta,
):
    from concourse.bass_utils import run_bass_kernel_spmd

    global _built
    if _built is None:
        _built = _build()
    nc = _built

    import ml_dtypes

    bf16 = ml_dtypes.bfloat16
    fp8 = ml_dtypes.float8_e4m3
    x = np.asarray(history_items, dtype=np.float32)
    mask = np.asarray(sequence_mask)
    f = lambda a: np.ascontiguousarray(np.asarray(a, dtype=np.float32))
    fb = lambda a: np.ascontiguousarray(np.asarray(a, dtype=np.float32).astype(bf16))
    f8 = lambda a: np.ascontiguousarray(np.asarray(a, dtype=np.float32).astype(fp8))

    common = {}
    for wname, W in (("wq", Wq), ("wk", Wk), ("wv", Wv), ("wo", Wo)):
        c0, c1 = _stripe_w(f(np.asarray(W).T * WS))
        common[f"{wname}0"] = f8(c0)
        common[f"{wname}1"] = f8(c1)
    common["bqs"] = f(np.asarray(bq).reshape(DB, P).T)
    common["bks"] = f(np.asarray(bk).reshape(DB, P).T)
    k_idx = np.arange(2 * H)[:, None]
    hb_idx = np.repeat(np.arange(SB), P)[None, :]
    c1_idx = np.tile((np.arange(P) >= 64).astype(np.int64), SB)[None, :]
    # fp8 ctx scale (x8) folded into the selector values
    common["pairsel"] = fb(
        CTX8 * (k_idx == 2 * hb_idx + c1_idx).astype(np.float32)
    )
    tri = np.where(
        np.arange(P)[None, :] >= np.arange(P)[:, None], 1.0, 0.0
    ).astype(np.float32)
    common["causal"] = f8(tri)
    common["causx"] = f8(np.concatenate([np.zeros((P, P), np.float32), tri], axis=1))
    # attn-output bias bv contributes bv @ Wo.T (constant over s) -> fold into residual
    bo_row = (
        np.asarray(bo, dtype=np.float64)
        + np.asarray(bv, dtype=np.float64) @ np.asarray(Wo, dtype=np.float64).T
    ).astype(np.float32)

    in_maps = []
    for b in range(N_CORES):
        xT = f(x[b].T * XS).astype(fp8).reshape(DB, P, S).transpose(1, 0, 2)
        pm = (mask[b] != 0).astype(np.float32)
        sx = x[b].astype(np.float64).sum(axis=0)
        sumv = ((sx @ np.asarray(Wv, dtype=np.float64).T) / 1024.0).astype(np.float32)
        # rows with an empty valid-causal window (reference: uniform over all keys)
        bad = np.cumsum(pm) == 0.0
        assert not bad[64:].any(), "bad rows beyond q=64 unsupported"
        badrow = np.zeros((H, 64), np.float32)
        badrow[:, :] = 1024.0 * bad[0:64][None, :]
        sumvr = np.zeros((1, H * (DK + 1)), np.float32)
        sumvr[0, :] = np.concatenate(
            [np.concatenate([sumv[h * DK : (h + 1) * DK], [0.0]]) for h in range(H)]
        )
        in_maps.append(
            {
                **common,
                "xt_lo": np.ascontiguousarray(xT[:, 0:4, :]),
                "xt_hi": np.ascontiguousarray(xT[:, 4:8, :]),
                "xr": f(x[b] + bo_row[None, :]),
                "padm": f(pm.reshape(SB, P).T),
                "badrow": fb(badrow),
                "sumvr": fb(sumvr),
            }
        )

    r = run_bass_kernel_spmd(nc, in_maps, core_ids=list(range(N_CORES)))
    out = np.stack([res["out"] for res in r.results]).astype(np.float32)

    g = np.asarray(ln_gamma, dtype=np.float32)
    be = np.asarray(ln_beta, dtype=np.float32)
    out = out * g[None, None, :] + be[None, None, :]
    return out.astype(np.float32)
